# revision 1
# baseline (speedup 1.0000x reference)
"""Longhorn SSM layer on 8 Trainium2 cores.

Sharding: core (b, j) with b in {0,1}, j in {0..3} handles batch b and
d_inner channel chunk [j*512, (j+1)*512).  The x_proj contraction needs all
d_inner channels, so partial x_dbl results are AllReduced across the 4 cores
of each batch.  The final out_proj partials are summed on the host.

The selective scan uses the DVE tensor_tensor_scan instruction
(state = a_t * state + b_t in fp32 along the free/time dim), one instruction
per (n, d-tile, t-half); a_t = 1 - dtv*kk is materialized in fp32 on ACT,
the other bulk tensors in bf16 for 2x DVE throughput.  y = sum_n S^n * q^n
is accumulated on the TensorEngine via identity matmuls into PSUM.
"""

import numpy as np
import ml_dtypes

import concourse.bacc as bacc
import concourse.bass as bass
import concourse.tile as tile
from concourse import mybir
from concourse.bass_utils import run_bass_kernel_spmd

F32 = mybir.dt.float32
BF16 = mybir.dt.bfloat16
AL = mybir.AluOpType
AF = mybir.ActivationFunctionType

BF = ml_dtypes.bfloat16


def build_module(L, DM, DI, DCH, NST, DTR, num_devices, use_collective):
    """Emit the per-core program.  All cores run the same program on
    different input data (pre-sliced weights)."""
    NG = DCH // 128          # d-tiles per core
    NK = DM // 128           # K-tiles for in_proj
    NO = DM // 128           # out_proj output tiles
    TQ = min(512, L)         # matmul moving-dim tile
    NTQ = L // TQ
    TH = L // 2              # t half (scan chunk)
    SQ = min(512, TH)        # matmul tile within a t-half
    NSQ = TH // SQ
    NR = DTR + 2 * NST       # x_proj rows
    PAD = 4

    nc = bacc.Bacc(
        "TRN2",
        target_bir_lowering=False,
        debug=False,
        enable_asserts=False,
        num_devices=num_devices,
    )

    # ---- I/O -------------------------------------------------------------
    hT_d = nc.dram_tensor("hT", [DM, L], F32, kind="ExternalInput")
    wx_d = nc.dram_tensor("wx", [128, NK * NG * 128], F32, kind="ExternalInput")
    wz_d = nc.dram_tensor("wz", [128, NK * NG * 128], F32, kind="ExternalInput")
    wo_d = nc.dram_tensor("wo", [128, NG * NO * 128], F32, kind="ExternalInput")
    dtw_d = nc.dram_tensor("dtw", [DTR, NG * 128], F32, kind="ExternalInput")
    xpw_d = nc.dram_tensor("xpw", [128, NG * NR], F32, kind="ExternalInput")
    pvec_d = nc.dram_tensor("pvec", [128, NG * 7], F32, kind="ExternalInput")
    ones_d = nc.dram_tensor("ones16", [NST, 128], F32, kind="ExternalInput")
    id_d = nc.dram_tensor("id128", [128, 128], BF16, kind="ExternalInput")
    outT_d = nc.dram_tensor("outT", [DM, L], F32, kind="ExternalOutput")

    # internal DRAM
    ccin_d = nc.dram_tensor("ccin", [NR, L], F32, kind="Internal")
    ccout_d = nc.dram_tensor("ccout", [NR, L], F32, kind="Internal")
    zsp_d = nc.dram_tensor("zsp", [128, NG, L], F32, kind="Internal")
    ygsp_d = nc.dram_tensor("ygsp", [128, NG, L], F32, kind="Internal")
    kbd = nc.dram_tensor("kbd", [NST, L], BF16, kind="Internal")
    qbd = nc.dram_tensor("qbd", [NST, L], BF16, kind="Internal")
    kkbd = nc.dram_tensor("kkbd", [NST, L], BF16, kind="Internal")

    groups = [[0, 1, 2, 3], [4, 5, 6, 7]] if num_devices == 8 else [[0]]

    with tile.TileContext(nc) as tc:
        with (
            tc.tile_pool(name="const", bufs=1) as constp,
            tc.tile_pool(name="persist", bufs=1) as pp,
        ):
            ones_sb = constp.tile([NST, 128], F32)
            nc.sync.dma_start(ones_sb, ones_d.ap())
            id_sb = constp.tile([128, 128], BF16)
            nc.sync.dma_start(id_sb, id_d.ap())
            pvec = constp.tile([128, NG, 7], F32)
            nc.sync.dma_start(pvec, pvec_d.ap().rearrange("p (g c) -> p g c", g=NG))

            # persistent SBUF through the scan phase
            xs = pp.tile([128, NG, L], F32)       # silu(conv(x))
            dtvb = pp.tile([128, NG, L], BF16)
            ub = pp.tile([128, NG, L], BF16)      # x*dtv

            # ---------------- phase A: in_proj / conv / x_dbl ------------
            with (
                tc.tile_pool(name="hw", bufs=1) as hwp,
                tc.tile_pool(name="xzbuf", bufs=1) as xzp,
                tc.tile_pool(name="psA", bufs=4, space="PSUM") as psA,
                tc.tile_pool(name="psX", bufs=1, space="PSUM") as psXp,
                tc.tile_pool(name="small", bufs=2) as smp,
            ):
                wx_sb = hwp.tile([128, NK, NG, 128], F32)
                wz_sb = hwp.tile([128, NK, NG, 128], F32)
                for k in range(NK):
                    nc.sync.dma_start(
                        wx_sb[:, k], wx_d.ap()[:, k * NG * 128:(k + 1) * NG * 128]
                        .rearrange("p (g m) -> p g m", g=NG))
                    nc.sync.dma_start(
                        wz_sb[:, k], wz_d.ap()[:, k * NG * 128:(k + 1) * NG * 128]
                        .rearrange("p (g m) -> p g m", g=NG))

                xpre = xzp.tile([128, NG, L + PAD], F32)
                for g in range(NG):
                    nc.vector.memset(xpre[:, g, 0:PAD], 0.0)

                for tq in range(NTQ):
                    ts = slice(tq * TQ, (tq + 1) * TQ)
                    hsb = hwp.tile([128, NK, TQ], F32, name="hsb", tag="hsb",
                                   bufs=2)
                    for k in range(NK):
                        nc.sync.dma_start(
                            hsb[:, k], hT_d.ap()[k * 128:(k + 1) * 128, ts])
                    for g in range(NG):
                        ps = psA.tile([128, TQ], F32, name="ps_xz", tag="psxz")
                        for k in range(NK):
                            nc.tensor.matmul(ps, wx_sb[:, k, g, :], hsb[:, k, :],
                                             start=(k == 0), stop=(k == NK - 1))
                        nc.scalar.copy(
                            xpre[:, g, PAD + tq * TQ: PAD + (tq + 1) * TQ], ps)
                        psz = psA.tile([128, TQ], F32, name="ps_z", tag="psxz")
                        for k in range(NK):
                            nc.tensor.matmul(psz, wz_sb[:, k, g, :], hsb[:, k, :],
                                             start=(k == 0), stop=(k == NK - 1))
                        zt = smp.tile([128, TQ], F32, name="zt")
                        nc.scalar.copy(zt, psz)
                        nc.sync.dma_start(zsp_d.ap()[:, g, ts], zt)

                # depthwise causal conv + silu   (tap j: w[:,j]*xpre[t-3+j])
                for g in range(NG):
                    a0 = xzp.tile([128, L], F32, name="cv0", tag="cva")
                    nc.vector.tensor_scalar(
                        a0, xpre[:, g, 1:1 + L], pvec[:, g, 0:1], None, op0=AL.mult)
                    a1 = xzp.tile([128, L], F32, name="cv1", tag="cvb")
                    nc.vector.scalar_tensor_tensor(
                        a1, xpre[:, g, 2:2 + L], pvec[:, g, 1:2], a0,
                        op0=AL.mult, op1=AL.add)
                    a2 = xzp.tile([128, L], F32, name="cv2", tag="cva")
                    nc.vector.scalar_tensor_tensor(
                        a2, xpre[:, g, 3:3 + L], pvec[:, g, 2:3], a1,
                        op0=AL.mult, op1=AL.add)
                    a3 = xzp.tile([128, L], F32, name="cv3", tag="cvb")
                    nc.vector.scalar_tensor_tensor(
                        a3, xpre[:, g, 4:4 + L], pvec[:, g, 3:4], a2,
                        op0=AL.mult, op1=AL.add)
                    # xs = silu(conv + conv_b) = (a3+cb) * sigmoid(a3+cb)
                    sg = xzp.tile([128, L], F32, name="sg", tag="cva")
                    nc.scalar.activation(sg, a3, AF.Sigmoid,
                                         bias=pvec[:, g, 6:7], scale=1.0)
                    nc.vector.scalar_tensor_tensor(
                        xs[:, g, :], a3, pvec[:, g, 6:7], sg,
                        op0=AL.add, op1=AL.mult)

                # partial x_dbl = xpw.T @ xs
                xpw_sb = hwp.tile([128, NG, NR], F32)
                for g in range(NG):
                    nc.sync.dma_start(
                        xpw_sb[:, g], xpw_d.ap()[:, g * NR:(g + 1) * NR])
                psX = psXp.tile([NR, L], F32)
                for tq in range(NTQ):
                    ts = slice(tq * TQ, (tq + 1) * TQ)
                    for g in range(NG):
                        nc.tensor.matmul(psX[:, ts], xpw_sb[:, g, :], xs[:, g, ts],
                                         start=(g == 0), stop=(g == NG - 1))
                xdp = xzp.tile([NR, L], F32)
                nc.scalar.copy(xdp, psX)
                nc.sync.dma_start(ccin_d.ap(), xdp)

            # ---------------- collective: sum partial x_dbl --------------
            if use_collective:
                nc.gpsimd.collective_compute(
                    "AllReduce", AL.add, replica_groups=groups,
                    ins=[ccin_d.ap()], outs=[ccout_d.ap()])
            else:
                nc.sync.dma_start(ccout_d.ap(), ccin_d.ap())

            # ---------------- phase A2: dt, dtv, u, rows ------------------
            with (
                tc.tile_pool(name="dtw", bufs=1) as dtwp,
                tc.tile_pool(name="psD", bufs=1, space="PSUM") as psDp,
                tc.tile_pool(name="psK", bufs=1, space="PSUM") as psKp,
                tc.tile_pool(name="rows", bufs=1) as rowp,
                tc.tile_pool(name="dtv", bufs=2) as dtvp,
            ):
                dtw_sb = dtwp.tile([DTR, NG, 128], F32)
                dtl_sb = rowp.tile([DTR, L], F32)
                nc.sync.dma_start(dtl_sb, ccout_d.ap()[0:DTR, :])
                krow = rowp.tile([NST, L], F32)
                nc.sync.dma_start(krow, ccout_d.ap()[DTR:DTR + NST, :])
                qrow = rowp.tile([NST, L], F32)
                nc.sync.dma_start(qrow, ccout_d.ap()[DTR + NST:NR, :])
                nc.sync.dma_start(
                    dtw_sb, dtw_d.ap().rearrange("p (g m) -> p g m", g=NG))

                kk = rowp.tile([NST, L], F32)
                nc.scalar.activation(kk, krow, AF.Square)
                kk1 = rowp.tile([NST, L], F32)
                nc.vector.tensor_scalar(kk1, kk, 1.0 / NST, None, op0=AL.add)
                kb16 = rowp.tile([NST, L], BF16)
                nc.vector.tensor_copy(kb16, krow)
                qb16 = rowp.tile([NST, L], BF16)
                nc.vector.tensor_copy(qb16, qrow)
                kkb16 = rowp.tile([NST, L], BF16)
                nc.vector.tensor_copy(kkb16, kk)
                nc.sync.dma_start(kbd.ap(), kb16)
                nc.sync.dma_start(qbd.ap(), qb16)
                nc.sync.dma_start(kkbd.ap(), kkb16)

                # SK1[t] = 1 + sum_n kk  broadcast to 128 partitions
                psK = psKp.tile([128, L], F32)
                for tq in range(NTQ):
                    ts = slice(tq * TQ, (tq + 1) * TQ)
                    nc.tensor.matmul(psK[:, ts], ones_sb, kk1[:, ts],
                                     start=True, stop=True)

                psD = psDp.tile([128, L], F32)
                for g in range(NG):
                    for tq in range(NTQ):
                        ts = slice(tq * TQ, (tq + 1) * TQ)
                        nc.tensor.matmul(psD[:, ts], dtw_sb[:, g, :],
                                         dtl_sb[:, ts], start=True, stop=True)
                    # E = exp(-(dt + b))
                    eg = dtvp.tile([128, L], F32, name="eg", tag="eg")
                    nc.scalar.activation(eg, psD, AF.Exp,
                                         bias=pvec[:, g, 4:5], scale=-1.0)
                    den = dtvp.tile([128, L], F32, name="den", tag="den")
                    nc.vector.tensor_tensor(den, eg, psK, op=AL.add)
                    dtv = dtvp.tile([128, L], F32, name="dtvf", tag="dtvf")
                    nc.vector.reciprocal_approx_fast(out=dtv, in_=den)
                    nc.vector.tensor_copy(dtvb[:, g, :], dtv)
                    nc.vector.tensor_tensor(ub[:, g, :], xs[:, g, :], dtv,
                                            op=AL.mult)

            # ---------------- phase B: the scan ---------------------------
            with (
                tc.tile_pool(name="psY", bufs=1, space="PSUM") as psYp,
                tc.tile_pool(name="bcast", bufs=2) as bcp,
                tc.tile_pool(name="scan", bufs=3) as scp,
                tc.tile_pool(name="carry", bufs=1) as cyp,
                tc.tile_pool(name="drain", bufs=2) as drp,
            ):
                GS = max(TH, 512)  # per-g PSUM stride: whole banks per group
                Y = psYp.tile([128, NG * GS], F32)
                sc = cyp.tile([128, NG * NST], F32)
                for th in range(2):
                    hs = slice(th * TH, (th + 1) * TH)
                    for n in range(NST):
                        kkb_t = bcp.tile([128, TH], BF16, name="kkb_t", tag="kkb")
                        nc.sync.dma_start(
                            kkb_t, kkbd.ap()[n:n + 1, hs].broadcast_to([128, TH]))
                        kb_t = bcp.tile([128, TH], BF16, name="kb_t", tag="kb")
                        nc.sync.dma_start(
                            kb_t, kbd.ap()[n:n + 1, hs].broadcast_to([128, TH]))
                        qb_t = bcp.tile([128, TH], BF16, name="qb_t", tag="qb")
                        nc.sync.dma_start(
                            qb_t, qbd.ap()[n:n + 1, hs].broadcast_to([128, TH]))
                        for g in range(NG):
                            col = g * NST + n
                            c_t = scp.tile([128, TH], BF16, name="c_t", tag="c")
                            nc.vector.tensor_tensor(c_t, dtvb[:, g, hs], kkb_t,
                                                    op=AL.mult)
                            a_t = scp.tile([128, TH], F32, name="a_t", tag="a")
                            nc.scalar.activation(a_t, c_t, AF.Identity,
                                                 bias=1.0, scale=-1.0)
                            b_t = scp.tile([128, TH], BF16, name="b_t", tag="b")
                            nc.gpsimd.tensor_tensor(b_t, ub[:, g, hs], kb_t,
                                                    op=AL.mult)
                            s_t = scp.tile([128, TH], BF16, name="s_t", tag="s")
                            init = 0.0 if th == 0 else sc[:, col:col + 1]
                            nc.vector.tensor_tensor_scan(
                                s_t, a_t, b_t, init, op0=AL.mult, op1=AL.add)
                            if th == 0:
                                nc.vector.tensor_copy(sc[:, col:col + 1],
                                                      s_t[:, TH - 1:TH])
                            p_t = scp.tile([128, TH], BF16, name="p_t", tag="p")
                            nc.vector.tensor_tensor(p_t, s_t, qb_t, op=AL.mult)
                            for h2 in range(NSQ):
                                nc.tensor.matmul(
                                    Y[:, g * GS + h2 * SQ: g * GS + (h2 + 1) * SQ],
                                    id_sb, p_t[:, h2 * SQ:(h2 + 1) * SQ],
                                    start=(n == 0), stop=(n == NST - 1))
                    # drain: ygated = (y + D*x) * silu(z)
                    for g in range(NG):
                        zt = drp.tile([128, TH], F32, name="zrl", tag="zrl")
                        nc.sync.dma_start(zt, zsp_d.ap()[:, g, hs])
                        szt = drp.tile([128, TH], F32, name="szt", tag="szt")
                        nc.scalar.activation(szt, zt, AF.Sigmoid)
                        zs2 = drp.tile([128, TH], F32, name="zs2", tag="zs2")
                        nc.gpsimd.tensor_tensor(zs2, zt, szt, op=AL.mult)
                        yb = drp.tile([128, TH], F32, name="yb", tag="yb")
                        nc.vector.scalar_tensor_tensor(
                            yb, xs[:, g, hs], pvec[:, g, 5:6],
                            Y[:, g * GS:g * GS + TH], op0=AL.mult, op1=AL.add)
                        yg = drp.tile([128, TH], F32, name="yg", tag="yg")
                        nc.vector.tensor_tensor(yg, yb, zs2, op=AL.mult)
                        nc.sync.dma_start(ygsp_d.ap()[:, g, hs], yg)

            # ---------------- phase C: out_proj ---------------------------
            with (
                tc.tile_pool(name="wo", bufs=1) as wop,
                tc.tile_pool(name="ygld", bufs=2) as ygp,
                tc.tile_pool(name="psO", bufs=4, space="PSUM") as psOp,
                tc.tile_pool(name="odr", bufs=3) as odp,
            ):
                wo_sb = wop.tile([128, NG, NO, 128], F32)
                for g in range(NG):
                    nc.sync.dma_start(
                        wo_sb[:, g],
                        wo_d.ap()[:, g * NO * 128:(g + 1) * NO * 128]
                        .rearrange("p (o m) -> p o m", o=NO))
                for tq in range(NTQ):
                    ts = slice(tq * TQ, (tq + 1) * TQ)
                    ygt = []
                    for g in range(NG):
                        t_ = ygp.tile([128, TQ], F32, name=f"ygt{g}", tag=f"ygt{g}")
                        nc.sync.dma_start(t_, ygsp_d.ap()[:, g, ts])
                        ygt.append(t_)
                    for o in range(NO):
                        po = psOp.tile([128, TQ], F32, name="po", tag="po")
                        for g in range(NG):
                            nc.tensor.matmul(po, wo_sb[:, g, o, :], ygt[g],
                                             start=(g == 0), stop=(g == NG - 1))
                        ot = odp.tile([128, TQ], F32, name="ot", tag="ot")
                        nc.scalar.copy(ot, po)
                        nc.sync.dma_start(outT_d.ap()[o * 128:(o + 1) * 128, ts], ot)

    nc.compile()
    return nc


# ----------------------------------------------------------------------------
# host-side packing
# ----------------------------------------------------------------------------

def pack_core_inputs(inputs, b, j, L, DM, DI, DCH, NST, DTR):
    NG = DCH // 128
    NK = DM // 128
    NO = DM // 128
    NR = DTR + 2 * NST
    ch = slice(j * DCH, (j + 1) * DCH)

    h = np.ascontiguousarray(np.asarray(inputs["hidden_states"], np.float32))
    ipw = np.asarray(inputs["in_proj_w"], np.float32)
    cw = np.asarray(inputs["conv_w"], np.float32).reshape(DI, 4)
    cb = np.asarray(inputs["conv_b"], np.float32)
    xpw = np.asarray(inputs["x_proj_w"], np.float32)
    dtw = np.asarray(inputs["dt_head_w"], np.float32)
    dtb = np.asarray(inputs["dt_head_b"], np.float32)
    opw = np.asarray(inputs["out_proj_w"], np.float32)
    D = np.asarray(inputs["D"], np.float32)

    hT = np.ascontiguousarray(h[b].T)                                   # [DM, L]
    wx = np.ascontiguousarray(
        ipw[ch].T.reshape(NK, 128, NG, 128).transpose(1, 0, 2, 3)
        .reshape(128, NK * NG * 128))
    wz = np.ascontiguousarray(
        ipw[DI + j * DCH: DI + (j + 1) * DCH].T
        .reshape(NK, 128, NG, 128).transpose(1, 0, 2, 3)
        .reshape(128, NK * NG * 128))
    wo = np.ascontiguousarray(
        opw[:, ch].T.reshape(NG, 128, NO, 128).transpose(1, 0, 2, 3)
        .reshape(128, NG * NO * 128))
    dtwp = np.ascontiguousarray(dtw[ch].T.reshape(DTR, NG * 128))
    xpwp = np.ascontiguousarray(
        xpw[:, ch].T.reshape(NG, 128, NR).transpose(1, 0, 2)
        .reshape(128, NG * NR))

    pv = np.zeros((128, NG, 7), np.float32)
    for g in range(NG):
        rows = slice(j * DCH + g * 128, j * DCH + (g + 1) * 128)
        pv[:, g, 0:4] = cw[rows]
        pv[:, g, 4] = -dtb[rows]
        pv[:, g, 5] = D[rows]
        pv[:, g, 6] = cb[rows]
    pvec = np.ascontiguousarray(pv.reshape(128, NG * 7))

    return {
        "hT": hT,
        "wx": wx,
        "wz": wz,
        "wo": wo,
        "dtw": dtwp,
        "xpw": xpwp,
        "pvec": pvec,
        "ones16": np.ones((NST, 128), np.float32),
        "id128": np.eye(128, dtype=np.float32).astype(BF),
    }


_CACHE = {}


def _get_module(key, *args, **kw):
    if key not in _CACHE:
        _CACHE[key] = build_module(*args, **kw)
    return _CACHE[key]


def run(inputs, trace=False, trace_cores=None):
    L, DM, DI = 2048, 1024, 2048
    DCH, NST, DTR = 512, 16, 64
    nc = _get_module("full", L, DM, DI, DCH, NST, DTR, 8, True)
    in_maps = []
    for core in range(8):
        b, j = divmod(core, 4)
        in_maps.append(pack_core_inputs(inputs, b, j, L, DM, DI, DCH, NST, DTR))
    res = run_bass_kernel_spmd(
        nc, in_maps, core_ids=list(range(8)), trace=trace,
        trace_cores=trace_cores)
    outs = [r["outT"] for r in res.results]
    full = np.empty((2, L, DM), np.float32)
    for b in range(2):
        acc = outs[4 * b].astype(np.float64)
        for j in range(1, 4):
            acc = acc + outs[4 * b + j]
        full[b] = acc.T.astype(np.float32)
    return full, res


def kernel(**inputs) -> np.ndarray:
    out, _ = run(inputs, trace=False)
    return out



# revision 4
# speedup vs baseline: 1.2898x; 1.2898x over previous
"""Longhorn SSM layer on 8 Trainium2 cores — chunked-pipeline version.

Sharding: core (b, j) with b in {0,1}, j in {0..3} handles batch b and
d_inner channel chunk [j*512, (j+1)*512).  x_proj partials are AllReduced
across the 4 cores of each batch (per 512-column chunk); out_proj partials
are summed on the host.

Layout per core: L=2048 is processed in 4 chunks of 512.  Per chunk:
  A: in_proj (bf16 matmuls) -> xpre (pre-conv x, bf16) and z
  B: causal depthwise conv via 4 ACT per-partition-scale taps + adds, silu
  C: x_proj partial (bf16 matmul) -> DRAM -> AllReduce (per chunk)
  D: dt head matmul, dtv = -1/(1+Sum kk+exp(-(dt+b))) (negated to fold the
     later sign), ub = -x*dtv, k/q/kk rows stored for broadcast
  E: for n in 16, g in 4: c'=dtvn*kk_n (DVE), a=1+c' (ACT), b'=ubn*k_n
     (GpSimd), s=tensor_tensor_scan(a,b') (DVE, fp32 state), p'=s*q_n,
     Y'[g] -= ... accumulated via identity matmuls in PSUM (Y' = -y)
  F: yg = (D*x - Y') * silu(z)
  G: out_proj partial (bf16) -> fp32 out

Elementwise work is statically balanced between DVE (fast, also owns the
serial scans) and GpSimd (slow); ACT absorbs all activation/copy work.
"""

import numpy as np
import ml_dtypes

import concourse.bacc as bacc
import concourse.bass as bass
import concourse.tile as tile
from concourse import mybir
from concourse.bass_utils import run_bass_kernel_spmd

F32 = mybir.dt.float32
BF16 = mybir.dt.bfloat16
AL = mybir.AluOpType
AF = mybir.ActivationFunctionType

BF = ml_dtypes.bfloat16

L, DM, DI = 2048, 1024, 2048
DCH, NST, DTR = 512, 16, 64
NG = DCH // 128      # 4
NK = DM // 128       # 8
NO = DM // 128       # 8
NR = DTR + 2 * NST   # 96
CH = 512             # chunk length
NCH = L // CH        # 4
PAD = 4


def build_module(num_devices=8):
    nc = bacc.Bacc(
        "TRN2",
        target_bir_lowering=False,
        debug=False,
        enable_asserts=False,
        num_devices=num_devices,
    )

    # ---- I/O -------------------------------------------------------------
    hT_d = nc.dram_tensor("hT", [DM, L], BF16, kind="ExternalInput")
    wx_d = nc.dram_tensor("wx", [128, NK * NG * 128], BF16, kind="ExternalInput")
    wz_d = nc.dram_tensor("wz", [128, NK * NG * 128], BF16, kind="ExternalInput")
    wo_d = nc.dram_tensor("wo", [128, NG * NO * 128], BF16, kind="ExternalInput")
    dtw_d = nc.dram_tensor("dtw", [DTR, NG * 128], BF16, kind="ExternalInput")
    xpw_d = nc.dram_tensor("xpw", [128, NG * NR], BF16, kind="ExternalInput")
    pvec_d = nc.dram_tensor("pvec", [128, NG * 8], F32, kind="ExternalInput")
    nones_d = nc.dram_tensor("nones16", [NST, 128], BF16, kind="ExternalInput")
    id_d = nc.dram_tensor("id128", [128, 128], BF16, kind="ExternalInput")
    outT_d = nc.dram_tensor("outT", [DM, L], F32, kind="ExternalOutput")

    # internal DRAM
    ccin_d = [nc.dram_tensor(f"ccin{c}", [NR, CH], F32, kind="Internal")
              for c in range(NCH)]
    ccout_d = [nc.dram_tensor(f"ccout{c}", [NR, CH], F32, kind="Internal")
               for c in range(NCH)]
    kqk_d = nc.dram_tensor("kqk", [3 * NST, L], BF16, kind="Internal")

    groups = [[0, 1, 2, 3], [4, 5, 6, 7]] if num_devices == 8 else [[0]]

    with tile.TileContext(nc) as tc:
        with (
            tc.tile_pool(name="const", bufs=1) as constp,
            tc.tile_pool(name="wts", bufs=1) as wp,
            tc.tile_pool(name="persist", bufs=1) as pp,
            tc.tile_pool(name="chunk", bufs=2) as cp,
            tc.tile_pool(name="small", bufs=2) as sp,
            tc.tile_pool(name="scan", bufs=3) as sc,
            tc.tile_pool(name="drain", bufs=2) as dr,
            tc.tile_pool(name="psA", bufs=2, space="PSUM") as psA,
            tc.tile_pool(name="psB", bufs=2, space="PSUM") as psB,
            tc.tile_pool(name="psY", bufs=1, space="PSUM") as psY,
        ):
            # ---------------- constants / weights ------------------------
            id_sb = constp.tile([128, 128], BF16)
            nc.sync.dma_start(id_sb, id_d.ap())
            nones_sb = constp.tile([NST, 128], BF16)
            nc.sync.dma_start(nones_sb, nones_d.ap())
            pvec = constp.tile([128, NG, 8], F32)
            nc.sync.dma_start(pvec, pvec_d.ap().rearrange("p (g c) -> p g c", g=NG))

            wx_sb = wp.tile([128, NK, NG, 128], BF16)
            wz_sb = wp.tile([128, NK, NG, 128], BF16)
            for k in range(NK):
                nc.sync.dma_start(
                    wx_sb[:, k], wx_d.ap()[:, k * NG * 128:(k + 1) * NG * 128]
                    .rearrange("p (g m) -> p g m", g=NG))
                nc.sync.dma_start(
                    wz_sb[:, k], wz_d.ap()[:, k * NG * 128:(k + 1) * NG * 128]
                    .rearrange("p (g m) -> p g m", g=NG))
            wo_sb = wp.tile([128, NG, NO, 128], BF16)
            for g in range(NG):
                nc.sync.dma_start(
                    wo_sb[:, g], wo_d.ap()[:, g * NO * 128:(g + 1) * NO * 128]
                    .rearrange("p (o m) -> p o m", o=NO))
            xpw_sb = wp.tile([128, NG, NR], BF16)
            for g in range(NG):
                nc.sync.dma_start(xpw_sb[:, g], xpw_d.ap()[:, g * NR:(g + 1) * NR])
            dtw_sb = wp.tile([DTR, NG, 128], BF16)
            nc.sync.dma_start(dtw_sb, dtw_d.ap().rearrange("p (g m) -> p g m", g=NG))

            # persistent
            xpre = pp.tile([128, NG, L + PAD], BF16)
            for g in range(NG):
                nc.vector.memset(xpre[:, g, 0:PAD], 0.0)
            carry = pp.tile([128, NST * NG], F32)
            inv_nst = pp.tile([NST, 1], F32)
            nc.vector.memset(inv_nst, 1.0 / NST)

            ctx = {}   # per-chunk tiles

            # ---------------- phases --------------------------------------
            def phase_A(ch):
                ts = slice(ch * CH, (ch + 1) * CH)
                hsb = cp.tile([128, NK, CH], BF16, name="hsb", tag="hsb", bufs=2)
                for k in range(NK):
                    nc.sync.dma_start(hsb[:, k], hT_d.ap()[k * 128:(k + 1) * 128, ts])
                zb = cp.tile([128, NG, CH], BF16, name="zb", tag="zb", bufs=2)
                for g in range(NG):
                    ps = psA.tile([128, CH], F32, name="psx", tag="mmA", bufs=2)
                    for k in range(NK):
                        nc.tensor.matmul(ps, wx_sb[:, k, g, :], hsb[:, k],
                                         start=(k == 0), stop=(k == NK - 1))
                    nc.scalar.activation(
                        xpre[:, g, PAD + ch * CH: PAD + (ch + 1) * CH], ps,
                        AF.Identity)
                    psz = psA.tile([128, CH], F32, name="psz", tag="mmA", bufs=2)
                    for k in range(NK):
                        nc.tensor.matmul(psz, wz_sb[:, k, g, :], hsb[:, k],
                                         start=(k == 0), stop=(k == NK - 1))
                    nc.scalar.activation(zb[:, g], psz, AF.Identity)
                ctx[("zb", ch)] = zb

            def phase_B(ch):
                xs = cp.tile([128, NG, CH], BF16, name="xs", tag="xs", bufs=2)
                base = PAD + ch * CH - 3
                for g in range(NG):
                    m = []
                    for j in range(4):
                        mj = sc.tile([128, CH], BF16, name=f"m{j}", tag=f"cv{j}",
                                     bufs=2)
                        nc.scalar.activation(
                            mj, xpre[:, g, base + j: base + j + CH], AF.Identity,
                            scale=pvec[:, g, j:j + 1],
                            bias=(pvec[:, g, 6:7] if j == 3 else 0.0))
                        m.append(mj)
                    t01 = sc.tile([128, CH], BF16, name="t01", tag="t01", bufs=2)
                    nc.vector.tensor_tensor(t01, m[0], m[1], op=AL.add)
                    t23 = sc.tile([128, CH], BF16, name="t23", tag="t23", bufs=2)
                    nc.vector.tensor_tensor(t23, m[2], m[3], op=AL.add)
                    cv = sc.tile([128, CH], BF16, name="cv", tag="cv", bufs=2)
                    nc.vector.tensor_tensor(cv, t01, t23, op=AL.add)
                    sg = sc.tile([128, CH], BF16, name="sg", tag="sgc", bufs=2)
                    nc.scalar.activation(sg, cv, AF.Sigmoid)
                    nc.vector.tensor_tensor(xs[:, g], cv, sg, op=AL.mult)
                ctx[("xs", ch)] = xs

            def phase_C(ch):
                xs = ctx[("xs", ch)]
                psX = psB.tile([NR, CH], F32, name="psX", tag="mmB", bufs=2)
                for g in range(NG):
                    nc.tensor.matmul(psX, xpw_sb[:, g, :], xs[:, g],
                                     start=(g == 0), stop=(g == NG - 1))
                xdp = sp.tile([NR, CH], F32, name="xdp", tag="xdp", bufs=2)
                nc.scalar.activation(xdp, psX, AF.Identity)
                nc.sync.dma_start(ccin_d[ch].ap(), xdp)

            def trigger(ch):
                nc.gpsimd.collective_compute(
                    "AllReduce", AL.add, replica_groups=groups,
                    ins=[ccin_d[ch].ap()], outs=[ccout_d[ch].ap()])

            def phase_D(ch):
                ts = slice(ch * CH, (ch + 1) * CH)
                xs = ctx[("xs", ch)]
                rows = sp.tile([NR, CH], F32, name="rows", tag="rows", bufs=2)
                nc.sync.dma_start(rows, ccout_d[ch].ap())
                rows_b = sp.tile([NR, CH], BF16, name="rows_b", tag="rowsb",
                                 bufs=2)
                nc.vector.tensor_copy(rows_b, rows)
                kkb = sp.tile([NST, CH], BF16, name="kkb", tag="kkb", bufs=2)
                nc.scalar.activation(kkb, rows_b[DTR:DTR + NST], AF.Square)
                kk1 = sp.tile([NST, CH], BF16, name="kk1", tag="kk1", bufs=2)
                nc.scalar.activation(kk1, kkb, AF.Identity, bias=inv_nst)
                nc.sync.dma_start(kqk_d.ap()[0:NST, ts], kkb)
                nc.sync.dma_start(kqk_d.ap()[NST:2 * NST, ts],
                                  rows_b[DTR:DTR + NST])
                nc.sync.dma_start(kqk_d.ap()[2 * NST:3 * NST, ts],
                                  rows_b[DTR + NST:NR])
                psK = psB.tile([128, CH], F32, name="psK", tag="mmB", bufs=2)
                nc.tensor.matmul(psK, nones_sb, kk1, start=True, stop=True)
                psKb = sp.tile([128, CH], BF16, name="psKb", tag="psKb", bufs=2)
                nc.scalar.activation(psKb, psK, AF.Identity)

                dtvb = cp.tile([128, NG, CH], BF16, name="dtvb", tag="dtvb",
                               bufs=2)
                ubn = cp.tile([128, NG, CH], BF16, name="ubn", tag="ubn", bufs=2)
                for g in range(NG):
                    psD = psB.tile([128, CH], F32, name="psD", tag="mmB", bufs=2)
                    nc.tensor.matmul(psD, dtw_sb[:, g, :], rows_b[0:DTR],
                                     start=True, stop=True)
                    eg = sp.tile([128, CH], BF16, name="eg", tag="eg", bufs=2)
                    nc.scalar.activation(eg, psD, AF.Exp,
                                         bias=pvec[:, g, 4:5], scale=-1.0)
                    den = sp.tile([128, CH], F32, name="den", tag="den", bufs=2)
                    nc.vector.tensor_tensor(den, psKb, eg, op=AL.subtract)
                    dtvf = sp.tile([128, CH], F32, name="dtvf", tag="dtvf",
                                   bufs=2)
                    nc.vector.reciprocal_approx_fast(out=dtvf, in_=den)
                    nc.scalar.activation(dtvb[:, g], dtvf, AF.Identity)
                    nc.vector.tensor_tensor(ubn[:, g], xs[:, g], dtvb[:, g],
                                            op=AL.mult)
                ctx[("dtvb", ch)] = dtvb
                ctx[("ubn", ch)] = ubn

            def phase_E(ch, n_lo, n_hi):
                ts = slice(ch * CH, (ch + 1) * CH)
                dtvb = ctx[("dtvb", ch)]
                ubn = ctx[("ubn", ch)]
                Y = ctx[("Y", ch)]
                for n in range(n_lo, n_hi):
                    kkt = sc.tile([128, CH], BF16, name="kkt", tag="kkt", bufs=2)
                    nc.sync.dma_start(
                        kkt, kqk_d.ap()[n:n + 1, ts].broadcast_to([128, CH]))
                    kt = sc.tile([128, CH], BF16, name="kt", tag="kt", bufs=2)
                    nc.sync.dma_start(
                        kt, kqk_d.ap()[NST + n:NST + n + 1, ts]
                        .broadcast_to([128, CH]))
                    qt = sc.tile([128, CH], BF16, name="qt", tag="qt", bufs=2)
                    nc.sync.dma_start(
                        qt, kqk_d.ap()[2 * NST + n:2 * NST + n + 1, ts]
                        .broadcast_to([128, CH]))
                    for g in range(NG):
                        col = n * NG + g
                        cpr = sc.tile([128, CH], BF16, name="cpr", tag="c",
                                      bufs=3)
                        nc.vector.tensor_tensor(cpr, dtvb[:, g], kkt, op=AL.mult)
                        a_t = sc.tile([128, CH], BF16, name="a_t", tag="a",
                                      bufs=3)
                        nc.scalar.activation(a_t, cpr, AF.Identity, bias=1.0)
                        b_t = sc.tile([128, CH], BF16, name="b_t", tag="b",
                                      bufs=3)
                        nc.gpsimd.tensor_tensor(b_t, ubn[:, g], kt, op=AL.mult)
                        s_t = sc.tile([128, CH], BF16, name="s_t", tag="s",
                                      bufs=3)
                        init = 0.0 if ch == 0 else carry[:, col:col + 1]
                        nc.vector.tensor_tensor_scan(s_t, a_t, b_t, init,
                                                     op0=AL.mult, op1=AL.add)
                        if ch < NCH - 1:
                            nc.scalar.copy(carry[:, col:col + 1],
                                           s_t[:, CH - 1:CH])
                        p_t = sc.tile([128, CH], BF16, name="p_t", tag="p",
                                      bufs=3)
                        if col % 4 == 0:
                            nc.vector.tensor_tensor(p_t, s_t, qt, op=AL.mult)
                        else:
                            nc.gpsimd.tensor_tensor(p_t, s_t, qt, op=AL.mult)
                        nc.tensor.matmul(Y[g], id_sb, p_t,
                                         start=(n == 0), stop=(n == NST - 1))

            def alloc_Y(ch):
                Y = [psY.tile([128, CH], F32, name=f"y{g}", tag=f"y{g}", bufs=1)
                     for g in range(NG)]
                ctx[("Y", ch)] = Y

            def phase_F(ch):
                xs = ctx[("xs", ch)]
                zb = ctx[("zb", ch)]
                Y = ctx[("Y", ch)]
                ygs = []
                for g in range(NG):
                    zsig = dr.tile([128, CH], BF16, name="zsig", tag="zsig",
                                   bufs=2)
                    nc.scalar.activation(zsig, zb[:, g], AF.Sigmoid)
                    zs2 = dr.tile([128, CH], BF16, name="zs2", tag="zs2", bufs=2)
                    nc.vector.tensor_tensor(zs2, zb[:, g], zsig, op=AL.mult)
                    yb = dr.tile([128, CH], BF16, name="yb", tag="yb", bufs=2)
                    nc.vector.scalar_tensor_tensor(
                        yb, xs[:, g], pvec[:, g, 5:6], Y[g],
                        op0=AL.mult, op1=AL.subtract)
                    yg = dr.tile([128, CH], BF16, name="yg", tag=f"yg{g}",
                                 bufs=2)
                    nc.vector.tensor_tensor(yg, yb, zs2, op=AL.mult)
                    ygs.append(yg)
                ctx[("ygs", ch)] = ygs

            def phase_G(ch):
                ts = slice(ch * CH, (ch + 1) * CH)
                ygs = ctx[("ygs", ch)]
                for o in range(NO):
                    po = psB.tile([128, CH], F32, name="po", tag="mmB", bufs=2)
                    for g in range(NG):
                        nc.tensor.matmul(po, wo_sb[:, g, o, :], ygs[g],
                                         start=(g == 0), stop=(g == NG - 1))
                    ot = sp.tile([128, CH], F32, name="ot", tag="ot", bufs=3)
                    nc.scalar.activation(ot, po, AF.Identity)
                    nc.sync.dma_start(outT_d.ap()[o * 128:(o + 1) * 128, ts], ot)

            # ---------------- emission (software pipeline) ----------------
            phase_A(0)
            phase_B(0)
            phase_C(0)
            trigger(0)
            phase_D(0)
            for ch in range(NCH):
                alloc_Y(ch)
                phase_E(ch, 0, 2)
                if ch + 1 < NCH:
                    phase_A(ch + 1)
                    phase_B(ch + 1)
                    phase_C(ch + 1)
                phase_E(ch, 2, 4)
                if ch + 1 < NCH:
                    trigger(ch + 1)
                phase_E(ch, 4, 10)
                if ch + 1 < NCH:
                    phase_D(ch + 1)
                phase_E(ch, 10, NST)
                phase_F(ch)
                phase_G(ch)

    nc.compile()
    return nc


# ----------------------------------------------------------------------------
# host-side packing
# ----------------------------------------------------------------------------

def pack_core_inputs(inputs, b, j):
    ch = slice(j * DCH, (j + 1) * DCH)

    h = np.asarray(inputs["hidden_states"], np.float32)
    ipw = np.asarray(inputs["in_proj_w"], np.float32)
    cw = np.asarray(inputs["conv_w"], np.float32).reshape(DI, 4)
    cb = np.asarray(inputs["conv_b"], np.float32)
    xpw = np.asarray(inputs["x_proj_w"], np.float32)
    dtw = np.asarray(inputs["dt_head_w"], np.float32)
    dtb = np.asarray(inputs["dt_head_b"], np.float32)
    opw = np.asarray(inputs["out_proj_w"], np.float32)
    D = np.asarray(inputs["D"], np.float32)

    hT = np.ascontiguousarray(h[b].T).astype(BF)
    wx = np.ascontiguousarray(
        ipw[ch].T.reshape(NK, 128, NG, 128).transpose(1, 0, 2, 3)
        .reshape(128, NK * NG * 128)).astype(BF)
    wz = np.ascontiguousarray(
        ipw[DI + j * DCH: DI + (j + 1) * DCH].T
        .reshape(NK, 128, NG, 128).transpose(1, 0, 2, 3)
        .reshape(128, NK * NG * 128)).astype(BF)
    wo = np.ascontiguousarray(
        opw[:, ch].T.reshape(NG, 128, NO, 128).transpose(1, 0, 2, 3)
        .reshape(128, NG * NO * 128)).astype(BF)
    dtwp = np.ascontiguousarray(dtw[ch].T.reshape(DTR, NG * 128)).astype(BF)
    xpwp = np.ascontiguousarray(
        xpw[:, ch].T.reshape(NG, 128, NR).transpose(1, 0, 2)
        .reshape(128, NG * NR)).astype(BF)

    pv = np.zeros((128, NG, 8), np.float32)
    for g in range(NG):
        rows = slice(j * DCH + g * 128, j * DCH + (g + 1) * 128)
        pv[:, g, 0:4] = cw[rows]
        pv[:, g, 4] = -dtb[rows]
        pv[:, g, 5] = D[rows]
        pv[:, g, 6] = cb[rows]
    pvec = np.ascontiguousarray(pv.reshape(128, NG * 8))

    return {
        "hT": hT,
        "wx": wx,
        "wz": wz,
        "wo": wo,
        "dtw": dtwp,
        "xpw": xpwp,
        "pvec": pvec,
        "nones16": np.full((NST, 128), -1.0, np.float32).astype(BF),
        "id128": np.eye(128, dtype=np.float32).astype(BF),
    }


_CACHE = {}


def _get_module(key, *args, **kw):
    if key not in _CACHE:
        _CACHE[key] = build_module(*args, **kw)
    return _CACHE[key]


def run(inputs, trace=False, trace_cores=None):
    nc = _get_module("full", 8)
    in_maps = []
    for core in range(8):
        b, j = divmod(core, 4)
        in_maps.append(pack_core_inputs(inputs, b, j))
    res = run_bass_kernel_spmd(
        nc, in_maps, core_ids=list(range(8)), trace=trace,
        trace_cores=trace_cores)
    outs = [r["outT"] for r in res.results]
    full = np.empty((2, L, DM), np.float32)
    for b in range(2):
        acc = outs[4 * b].astype(np.float64)
        for j in range(1, 4):
            acc = acc + outs[4 * b + j]
        full[b] = acc.T.astype(np.float32)
    return full, res


def kernel(**inputs) -> np.ndarray:
    out, _ = run(inputs, trace=False)
    return out


# revision 32
# speedup vs baseline: 1.6906x; 1.3107x over previous
"""Longhorn SSM layer on 8 Trainium2 cores — v3.

Sharding: core (b, j) handles batch b and d_inner channels [j*512,(j+1)*512).
x_proj partials AllReduced over each batch's 4 cores; out_proj partials
summed on the host.

Engine assignment (TRN2: DVE 2-src ops and ALL GpSimd ops serialize on the
shared SBUF port pair, so GpSimd does nothing):
  PE : in_proj / x_proj / dt / out_proj matmuls (bf16), depthwise conv as 4
       PSUM-accumulated diag-matmuls with shifted time slices, y = sum_n
       s*q via identity matmuls into PSUM.
  ACT: silu (native Silu), exp, square, all PSUM->SBUF copies/casts,
       a = 1 + c' (bias add), scan carry copies.
  DVE: the serial scans (fp32 state), c' = dtvn*kk_n, b' = ubn*k_n,
       p' = s*q_n (bf16 2x), dtv denominator + reciprocal, drain STT.

Negation folding: psK' = -(1+sum kk) via (-1)-matrix matmul, den' = psK'-eg
= -den, dtvn = 1/den' = -dtv, ubn = xs*dtvn = -x*dtv, a = 1 + dtvn*kk,
b' = ubn*k = -b, so the scan computes s' = -s, Y' = -y, and the drain uses
yb = D*x - Y' = D*x + y.

Time is processed in two halves of 1024 for the scan phase (prep in four
512 chunks feeding half-sized tiles); within a half the 16-state loop runs
over g-pairs so Y PSUM needs only 4 banks at a time.
"""

import numpy as np
import ml_dtypes

import concourse.bacc as bacc
import concourse.bass as bass
import concourse.tile as tile
from concourse import mybir
from concourse.bass_utils import run_bass_kernel_spmd

F32 = mybir.dt.float32
BF16 = mybir.dt.bfloat16
AL = mybir.AluOpType
AF = mybir.ActivationFunctionType

BF = ml_dtypes.bfloat16

L, DM, DI = 2048, 1024, 2048
DCH, NST, DTR = 512, 16, 64
NG = DCH // 128      # 4
NK = DM // 128       # 8
NO = DM // 128       # 8
NR = DTR + 2 * NST   # 96
CH = 512             # prep chunk
NCH = L // CH        # 4
HL = 1024            # scan half length
PAD = 4


def build_module(num_devices=8):
    nc = bacc.Bacc(
        "TRN2",
        target_bir_lowering=False,
        debug=False,
        enable_asserts=False,
        num_devices=num_devices,
    )

    # ---- I/O -------------------------------------------------------------
    hT_d = nc.dram_tensor("hT", [DM, L], BF16, kind="ExternalInput")
    wx_d = nc.dram_tensor("wx", [128, NK * NG * 128], BF16, kind="ExternalInput")
    wz_d = nc.dram_tensor("wz", [128, NK * NG * 128], BF16, kind="ExternalInput")
    wo_d = nc.dram_tensor("wo", [128, NG * NO * 128], BF16, kind="ExternalInput")
    wc_d = nc.dram_tensor("wc", [128, NG * 4 * 128], BF16, kind="ExternalInput")
    dtw_d = nc.dram_tensor("dtw", [DTR, NG * 128], BF16, kind="ExternalInput")
    xpw_d = nc.dram_tensor("xpw", [128, NG * NR], BF16, kind="ExternalInput")
    pvec_d = nc.dram_tensor("pvec", [128, NG * 8], F32, kind="ExternalInput")
    nones_d = nc.dram_tensor("nones16", [NST, 128], BF16, kind="ExternalInput")
    id_d = nc.dram_tensor("id128", [128, 128], BF16, kind="ExternalInput")
    outT_d = nc.dram_tensor("outT", [DM, L], F32, kind="ExternalOutput")
    outT2_d = nc.dram_tensor("outT2", [DM, L], F32, kind="ExternalOutput")
    outT3_d = nc.dram_tensor("outT3", [DM, L], F32, kind="ExternalOutput")

    ccin_d = [nc.dram_tensor(f"ccin{h}", [NR, HL], BF16, kind="Internal")
              for h in range(2)]
    ccout_d = [nc.dram_tensor(f"ccout{h}", [NR, HL], BF16, kind="Internal")
               for h in range(2)]
    kqk_d = nc.dram_tensor("kqk", [3 * NST, L], BF16, kind="Internal")

    groups = [[0, 1, 2, 3], [4, 5, 6, 7]] if num_devices == 8 else [[0]]

    with tile.TileContext(nc) as tc:
        with (
            tc.tile_pool(name="const", bufs=1) as constp,
            tc.tile_pool(name="wts", bufs=1) as wp,
            tc.tile_pool(name="persist", bufs=1) as pp,
            tc.tile_pool(name="half", bufs=2) as hp,
            tc.tile_pool(name="small", bufs=2) as sp,
            tc.tile_pool(name="scan", bufs=3) as sc,
            tc.tile_pool(name="drain", bufs=2) as dr,
            tc.tile_pool(name="psA", bufs=2, space="PSUM") as psA,
            tc.tile_pool(name="psB", bufs=2, space="PSUM") as psB,
            tc.tile_pool(name="psY", bufs=1, space="PSUM") as psY,
        ):
            # ---------------- constants / weights ------------------------
            id_sb = constp.tile([128, 128], BF16)
            nc.sync.dma_start(id_sb, id_d.ap())
            nones_sb = constp.tile([NST, 128], BF16)
            nc.sync.dma_start(nones_sb, nones_d.ap())
            pvec = constp.tile([128, NG, 8], F32)
            nc.sync.dma_start(pvec, pvec_d.ap().rearrange("p (g c) -> p g c", g=NG))
            inv_nst = constp.tile([NST, 1], F32)
            nc.vector.memset(inv_nst, 1.0 / NST)

            wx_sb = wp.tile([128, NK, NG, 128], BF16)
            wz_sb = wp.tile([128, NK, NG, 128], BF16)
            for k in range(NK):
                nc.sync.dma_start(
                    wx_sb[:, k], wx_d.ap()[:, k * NG * 128:(k + 1) * NG * 128]
                    .rearrange("p (g m) -> p g m", g=NG))
                nc.sync.dma_start(
                    wz_sb[:, k], wz_d.ap()[:, k * NG * 128:(k + 1) * NG * 128]
                    .rearrange("p (g m) -> p g m", g=NG))
            wo_sb = wp.tile([128, NG, NO, 128], BF16)
            for g in range(NG):
                nc.sync.dma_start(
                    wo_sb[:, g], wo_d.ap()[:, g * NO * 128:(g + 1) * NO * 128]
                    .rearrange("p (o m) -> p o m", o=NO))
            wc_sb = wp.tile([128, NG, 4, 128], BF16)
            for g in range(NG):
                nc.sync.dma_start(
                    wc_sb[:, g], wc_d.ap()[:, g * 4 * 128:(g + 1) * 4 * 128]
                    .rearrange("p (j m) -> p j m", j=4))
            xpw_sb = wp.tile([128, NG, NR], BF16)
            for g in range(NG):
                nc.sync.dma_start(xpw_sb[:, g], xpw_d.ap()[:, g * NR:(g + 1) * NR])
            dtw_sb = wp.tile([DTR, NG, 128], BF16)
            nc.sync.dma_start(dtw_sb, dtw_d.ap().rearrange("p (g m) -> p g m", g=NG))

            sbuf_pad = sp.tile([128, 512], BF16, name="sbuf_pad", tag="pad",
                               bufs=1)
            nc.vector.memset(sbuf_pad[:, 0:4], 0.0)
            xpre = pp.tile([128, NG, L + PAD], BF16)
            for g in range(NG):
                nc.vector.memset(xpre[:, g, 0:PAD], 0.0)
            carry = pp.tile([128, NST * NG], F32)

            ctx = {}

            # ------------- prep: in_proj + conv/silu + x_proj -------------
            def prep_x(ch):
                h = ch // 2
                ts = slice(ch * CH, (ch + 1) * CH)
                hs = slice((ch % 2) * CH, (ch % 2 + 1) * CH)
                if ch % 2 == 0:
                    ctx[("xs", h)] = hp.tile([128, NG, HL], BF16, name="xs",
                                             tag="xs", bufs=2)
                    ctx[("zs2", h)] = hp.tile([128, NG, HL], BF16, name="zs2",
                                              tag="zs2", bufs=2)
                xs = ctx[("xs", h)]
                hsb = sp.tile([128, NK, CH], BF16, name="hsb", tag="hsb", bufs=2)
                ctx[("hsb", ch)] = hsb
                for k in range(NK):
                    nc.sync.dma_start(hsb[:, k], hT_d.ap()[k * 128:(k + 1) * 128, ts])

                # During the head (ch<2) the Y PSUM banks are idle; borrow
                # them to deepen the in_proj ring (hides the copy latency).
                def prep_ps(idx):
                    if ch < 2 and idx % 2 == 1:
                        return psY.tile([128, CH], F32, name="psx",
                                        tag=f"y{(idx // 2) % 2}", bufs=1)
                    return psA.tile([128, CH], F32, name="psx", tag="mmA",
                                    bufs=2)

                for g in range(NG):
                    ps = prep_ps(g)
                    for k in range(NK):
                        nc.tensor.matmul(ps, wx_sb[:, k, g, :], hsb[:, k],
                                         start=(k == 0), stop=(k == NK - 1))
                    nc.scalar.activation(
                        xpre[:, g, PAD + ch * CH: PAD + (ch + 1) * CH], ps,
                        AF.Identity)
                # depthwise conv as 4 shifted diag matmuls + native silu
                for g in range(NG):
                    psC = prep_ps(g)
                    base = PAD + ch * CH - 3
                    for j in range(4):
                        nc.tensor.matmul(psC, wc_sb[:, g, j, :],
                                         xpre[:, g, base + j: base + j + CH],
                                         start=(j == 0), stop=(j == 3))
                    nc.scalar.activation(xs[:, g, hs], psC, AF.Silu,
                                         bias=pvec[:, g, 6:7])
                # x_proj partial
                psX = psB.tile([NR, CH], F32, name="psX", tag="mmB", bufs=2)
                for g in range(NG):
                    nc.tensor.matmul(psX, xpw_sb[:, g, :], xs[:, g, hs],
                                     start=(g == 0), stop=(g == NG - 1))
                xdp = sp.tile([NR, CH], BF16, name="xdp", tag="xdp", bufs=2)
                nc.scalar.activation(xdp, psX, AF.Identity)
                nc.sync.dma_start(ccin_d[h].ap()[:, hs], xdp)

            def prep_z(ch):
                h = ch // 2
                hs = slice((ch % 2) * CH, (ch % 2 + 1) * CH)
                zs2 = ctx[("zs2", h)]
                hsb = ctx[("hsb", ch)]
                for g in range(NG):
                    psz = psA.tile([128, CH], F32, name="psz", tag="mmA", bufs=2)
                    for k in range(NK):
                        nc.tensor.matmul(psz, wz_sb[:, k, g, :], hsb[:, k],
                                         start=(k == 0), stop=(k == NK - 1))
                    nc.scalar.activation(zs2[:, g, hs], psz, AF.Silu)

            def trigger(h):
                nc.gpsimd.collective_compute(
                    "AllReduce", AL.add, replica_groups=groups,
                    ins=[ccin_d[h].ap()], outs=[ccout_d[h].ap()])

            # ------------- A2: rows, du = [dtvn | ubn] (per 512 chunk) ----
            def phase_A2(ch):
                h = ch // 2
                ts = slice(ch * CH, (ch + 1) * CH)
                hs = slice((ch % 2) * CH, (ch % 2 + 1) * CH)
                xs = ctx[("xs", h)]
                if ch % 2 == 0:
                    # du[:, 0, g] = dtvn, du[:, 1, g] = ubn: adjacent planes so
                    # the scan phase multiplies both against [kk_n | k_n] in
                    # one fused DVE op per (n, g).
                    ctx[("du", h)] = hp.tile([128, 2, NG, HL], BF16, name="du",
                                             tag="du", bufs=2)
                du = ctx[("du", h)]
                rows = sp.tile([NR, CH], BF16, name="rows", tag="rows", bufs=2)
                nc.sync.dma_start(rows, ccout_d[h].ap()[:, hs])
                kkb = sp.tile([NST, CH], BF16, name="kkb", tag="kkb", bufs=2)
                nc.scalar.activation(kkb, rows[DTR:DTR + NST], AF.Square)
                kk1 = sp.tile([NST, CH], BF16, name="kk1", tag="kk1", bufs=2)
                nc.scalar.activation(kk1, kkb, AF.Identity, bias=inv_nst)
                nc.sync.dma_start(kqk_d.ap()[0:NST, ts], kkb)
                nc.sync.dma_start(kqk_d.ap()[NST:2 * NST, ts],
                                  rows[DTR:DTR + NST])
                nc.sync.dma_start(kqk_d.ap()[2 * NST:3 * NST, ts],
                                  rows[DTR + NST:NR])
                psK = psB.tile([128, CH], F32, name="psK", tag="mmB", bufs=2)
                nc.tensor.matmul(psK, nones_sb, kk1, start=True, stop=True)
                psKb = sp.tile([128, CH], BF16, name="psKb", tag="psKb", bufs=2)
                nc.scalar.activation(psKb, psK, AF.Identity)

                for g in range(NG):
                    psD = psB.tile([128, CH], F32, name="psD", tag="mmB",
                                   bufs=2)
                    nc.tensor.matmul(psD, dtw_sb[:, g, :], rows[0:DTR],
                                     start=True, stop=True)
                    eg = sp.tile([128, CH], BF16, name="eg", tag="eg", bufs=1)
                    nc.scalar.activation(eg, psD, AF.Exp,
                                         bias=pvec[:, g, 4:5], scale=-1.0)
                    den = sp.tile([128, CH], F32, name="den", tag="den", bufs=2)
                    nc.vector.tensor_tensor(den, psKb, eg, op=AL.subtract)
                    dtvf = sp.tile([128, CH], F32, name="dtvf", tag="dtvf",
                                   bufs=2)
                    nc.vector.reciprocal_approx_fast(out=dtvf, in_=den)
                    nc.scalar.activation(du[:, 0, g, hs], dtvf, AF.Identity)
                    nc.vector.tensor_tensor(du[:, 1, g, hs], xs[:, g, hs],
                                            du[:, 0, g, hs], op=AL.mult)

            # ------------- scan phase over one g-pair ---------------------
            def scan_pass(h, gs, hooks=None):
                hooks = hooks or {}
                ts = slice(h * HL, (h + 1) * HL)
                du = ctx[("du", h)]
                Y = {g: psY.tile([128, HL], F32, name=f"y{g}", tag=f"y{g % 2}",
                                 bufs=1) for g in gs}
                ctx[("Y", h, gs)] = Y
                bc = {}

                def load_bcast(n):
                    kkt = sc.tile([128, 2, HL], BF16, name="kkt", tag="kkt",
                                  bufs=2)
                    nc.sync.dma_start(
                        kkt[:, 0], kqk_d.ap()[n:n + 1, ts]
                        .broadcast_to([128, HL]))
                    nc.sync.dma_start(
                        kkt[:, 1], kqk_d.ap()[NST + n:NST + n + 1, ts]
                        .broadcast_to([128, HL]))
                    qt = sc.tile([128, HL], BF16, name="qt", tag="qt", bufs=3)
                    nc.sync.dma_start(
                        qt, kqk_d.ap()[2 * NST + n:2 * NST + n + 1, ts]
                        .broadcast_to([128, HL]))
                    bc[n] = (kkt, qt)

                load_bcast(0)
                pend = None  # (n, {g: (a, cb2, qt)})
                for n in range(NST + 1):
                    if n in hooks:
                        hooks[n]()
                    if n + 1 < NST:
                        load_bcast(n + 1)
                    if n < NST:
                        kkt, qt = bc.pop(n)
                        cur = {}
                        for g in gs:
                            cb2 = sc.tile([128, 2, HL], BF16, name="cb2",
                                          tag="cb", bufs=2)
                            nc.vector.tensor_tensor(cb2, du[:, :, g, :], kkt,
                                                    op=AL.mult)
                            a_t = sc.tile([128, HL], BF16, name="a_t", tag="a",
                                          bufs=2)
                            nc.scalar.activation(a_t, cb2[:, 0], AF.Identity,
                                                 bias=1.0)
                            cur[g] = (a_t, cb2, qt)
                    # retire previous n: scan, p', Y
                    if pend is not None:
                        pn, tiles = pend
                        for g in gs:
                            a_t, cb2, pqt = tiles[g]
                            col = pn * NG + g
                            s_t = sc.tile([128, HL], BF16, name="s_t", tag="s",
                                          bufs=2)
                            init = 0.0 if h == 0 else carry[:, col:col + 1]
                            nc.vector.tensor_tensor_scan(
                                s_t, a_t, cb2[:, 1], init,
                                op0=AL.mult, op1=AL.add)
                            if h == 0:
                                nc.scalar.copy(carry[:, col:col + 1],
                                               s_t[:, HL - 1:HL])
                            p_t = sc.tile([128, HL], BF16, name="p_t", tag="p",
                                          bufs=2)
                            nc.vector.tensor_tensor(p_t, s_t, pqt, op=AL.mult)
                            for q in range(2):
                                nc.tensor.matmul(
                                    Y[g][:, q * CH:(q + 1) * CH], id_sb,
                                    p_t[:, q * CH:(q + 1) * CH],
                                    start=(pn == 0), stop=(pn == NST - 1))
                    pend = (n, cur) if n < NST else None

            # ------------- drain + out_proj -------------------------------
            def drain(h, gs):
                xs = ctx[("xs", h)]
                zs2 = ctx[("zs2", h)]
                Y = ctx[("Y", h, gs)]
                for g in gs:
                    yb = dr.tile([128, HL], BF16, name="yb", tag="yb", bufs=2)
                    nc.vector.scalar_tensor_tensor(
                        yb, xs[:, g], pvec[:, g, 5:6], Y[g],
                        op0=AL.mult, op1=AL.subtract)
                    yg = dr.tile([128, HL], BF16, name="yg", tag=f"yg{g}",
                                 bufs=2)
                    nc.vector.tensor_tensor(yg, yb, zs2[:, g], op=AL.mult)
                    ctx[("yg", h, g)] = yg

            def out_proj(h, gs, dst, last=False):
                ygs = {g: ctx[("yg", h, g)] for g in gs}
                for q in range(2):
                    ts = slice(h * HL + q * CH, h * HL + (q + 1) * CH)
                    qs = slice(q * CH, (q + 1) * CH)
                    for o in range(NO):
                        if last:
                            # every PSUM bank is idle at the tail; cycle po
                            # over four rings so no group waits on a copy
                            idx = (q * NO + o) % 4
                            if idx == 0:
                                po = psY.tile([128, CH], F32, name="po",
                                              tag="y0", bufs=1)
                            elif idx == 1:
                                po = psY.tile([128, CH], F32, name="po",
                                              tag="y1", bufs=1)
                            elif idx == 2:
                                po = psA.tile([128, CH], F32, name="po",
                                              tag="mmA", bufs=2)
                            else:
                                po = psB.tile([128, CH], F32, name="po",
                                              tag="mmB", bufs=2)
                        else:
                            po = psB.tile([128, CH], F32, name="po", tag="mmB",
                                          bufs=2)
                        for g in gs:
                            nc.tensor.matmul(po, wo_sb[:, g, o, :], ygs[g][:, qs],
                                             start=(g == gs[0]),
                                             stop=(g == gs[-1]))
                        ot = sp.tile([128, CH], F32, name="ot", tag="ot", bufs=2)
                        if last:
                            nc.vector.tensor_copy(ot, po)
                        else:
                            nc.scalar.activation(ot, po, AF.Identity)
                        nc.sync.dma_start(dst.ap()[o * 128:(o + 1) * 128, ts],
                                          ot)

            # ---------------- emission ------------------------------------
            prep_x(0)
            prep_x(1)
            trigger(0)
            phase_A2(0)
            phase_A2(1)
            hooks0 = {1: lambda: prep_z(0), 3: lambda: prep_z(1),
                      5: lambda: prep_x(2), 8: lambda: prep_x(3),
                      10: lambda: trigger(1), 11: lambda: prep_z(2)}
            hooks1 = {1: lambda: prep_z(3), 3: lambda: phase_A2(2),
                      6: lambda: phase_A2(3)}
            scan_pass(0, (0, 1), hooks0)
            drain(0, (0, 1))
            out_proj(0, (0, 1), outT_d)
            scan_pass(0, (2, 3), hooks1)
            drain(0, (2, 3))
            out_proj(0, (2, 3), outT2_d)
            scan_pass(1, (0, 1))
            drain(1, (0, 1))
            out_proj(1, (0, 1), outT_d)
            scan_pass(1, (2, 3))
            drain(1, (2, 3))
            out_proj(1, (2, 3), outT2_d, last=True)

    nc.compile()
    return nc


# ----------------------------------------------------------------------------
# host-side packing
# ----------------------------------------------------------------------------

def pack_core_inputs(inputs, b, j):
    ch = slice(j * DCH, (j + 1) * DCH)

    h = np.asarray(inputs["hidden_states"], np.float32)
    ipw = np.asarray(inputs["in_proj_w"], np.float32)
    cw = np.asarray(inputs["conv_w"], np.float32).reshape(DI, 4)
    cb = np.asarray(inputs["conv_b"], np.float32)
    xpw = np.asarray(inputs["x_proj_w"], np.float32)
    dtw = np.asarray(inputs["dt_head_w"], np.float32)
    dtb = np.asarray(inputs["dt_head_b"], np.float32)
    opw = np.asarray(inputs["out_proj_w"], np.float32)
    D = np.asarray(inputs["D"], np.float32)

    hT = np.ascontiguousarray(h[b].T).astype(BF)
    wx = np.ascontiguousarray(
        ipw[ch].T.reshape(NK, 128, NG, 128).transpose(1, 0, 2, 3)
        .reshape(128, NK * NG * 128)).astype(BF)
    wz = np.ascontiguousarray(
        ipw[DI + j * DCH: DI + (j + 1) * DCH].T
        .reshape(NK, 128, NG, 128).transpose(1, 0, 2, 3)
        .reshape(128, NK * NG * 128)).astype(BF)
    wo = np.ascontiguousarray(
        opw[:, ch].T.reshape(NG, 128, NO, 128).transpose(1, 0, 2, 3)
        .reshape(128, NG * NO * 128)).astype(BF)
    dtwp = np.ascontiguousarray(dtw[ch].T.reshape(DTR, NG * 128)).astype(BF)
    xpwp = np.ascontiguousarray(
        xpw[:, ch].T.reshape(NG, 128, NR).transpose(1, 0, 2)
        .reshape(128, NG * NR)).astype(BF)
    # diag conv-tap matrices: wc[:, g, j, :] = diag(cw[rows(g), j])
    wc = np.zeros((128, NG, 4, 128), np.float32)
    for g in range(NG):
        rows = slice(j * DCH + g * 128, j * DCH + (g + 1) * 128)
        for t in range(4):
            np.fill_diagonal(wc[:, g, t, :], 0)
            wc[np.arange(128), g, t, np.arange(128)] = cw[rows, t]
    wcp = np.ascontiguousarray(wc.reshape(128, NG * 4 * 128)).astype(BF)

    pv = np.zeros((128, NG, 8), np.float32)
    for g in range(NG):
        rows = slice(j * DCH + g * 128, j * DCH + (g + 1) * 128)
        pv[:, g, 0:4] = cw[rows]
        pv[:, g, 4] = -dtb[rows]
        pv[:, g, 5] = D[rows]
        pv[:, g, 6] = cb[rows]
    pvec = np.ascontiguousarray(pv.reshape(128, NG * 8))

    return {
        "hT": hT,
        "wx": wx,
        "wz": wz,
        "wo": wo,
        "wc": wcp,
        "dtw": dtwp,
        "xpw": xpwp,
        "pvec": pvec,
        "nones16": np.full((NST, 128), -1.0, np.float32).astype(BF),
        "id128": np.eye(128, dtype=np.float32).astype(BF),
    }


_CACHE = {}


def _get_module(key, *args, **kw):
    if key not in _CACHE:
        _CACHE[key] = build_module(*args, **kw)
    return _CACHE[key]


def run(inputs, trace=False, trace_cores=None):
    nc = _get_module("full", 8)
    in_maps = []
    for core in range(8):
        b, j = divmod(core, 4)
        in_maps.append(pack_core_inputs(inputs, b, j))
    res = run_bass_kernel_spmd(
        nc, in_maps, core_ids=list(range(8)), trace=trace,
        trace_cores=trace_cores)
    full = np.empty((2, L, DM), np.float32)
    for b in range(2):
        acc = res.results[4 * b]["outT"].astype(np.float64)
        for j in range(4):
            r = res.results[4 * b + j]
            if j > 0:
                acc = acc + r["outT"]
            acc = acc + r["outT2"]
            # outT3 unused in the paired-pass layout
        full[b] = acc.T.astype(np.float32)
    return full, res


def kernel(**inputs) -> np.ndarray:
    out, _ = run(inputs, trace=False)
    return out
